# revision 13
# baseline (speedup 1.0000x reference)
import numpy as np
import concourse.bass as bass
import concourse.mybir as mybir
import concourse.tile as tile
from contextlib import ExitStack
from concourse import bacc

P = 128
L = 1024
E = 512
H = 8
D = 64
NB = 2
TT = L // P
EPO = E // P
FP32 = mybir.dt.float32
FP16 = mybir.dt.float16
AF = mybir.ActivationFunctionType
ALU = mybir.AluOpType


def host_constants():
    ident16 = np.eye(P, dtype=np.float16)
    ident32 = np.eye(P, dtype=np.float32)
    sel2 = np.zeros((P, H * D), np.float16)
    for h in range(H):
        sel2[32 * (h % 4), h * D:(h + 1) * D] = 1.0
    return ident16, ident32, sel2


def build(debug=False):
    nc = bacc.Bacc("TRN2", target_bir_lowering=False, debug=debug)
    q_d = nc.dram_tensor("q", [NB, L, E], FP32, kind="ExternalInput").ap()
    k_d = nc.dram_tensor("k", [NB, L, E], FP32, kind="ExternalInput").ap()
    v_d = nc.dram_tensor("v", [NB, L, E], FP32, kind="ExternalInput").ap()
    wq_d = nc.dram_tensor("Wq", [E, E], FP32, kind="ExternalInput").ap()
    wk_d = nc.dram_tensor("Wk", [E, E], FP32, kind="ExternalInput").ap()
    wv_d = nc.dram_tensor("Wv", [E, E], FP32, kind="ExternalInput").ap()
    wo_d = nc.dram_tensor("Wo", [E, E], FP32, kind="ExternalInput").ap()
    bo_d = nc.dram_tensor("bo_bcast", [P, E], FP32, kind="ExternalInput").ap()
    id16_d = nc.dram_tensor("ident16", [P, P], FP16, kind="ExternalInput").ap()
    id32_d = nc.dram_tensor("ident32", [P, P], FP32, kind="ExternalInput").ap()
    sel_d = nc.dram_tensor("sel2", [P, H * D], FP16, kind="ExternalInput").ap()
    out_d = nc.dram_tensor("out", [NB, L, E], FP32, kind="ExternalOutput").ap()
    x_d = {"q": q_d, "k": k_d, "v": v_d}

    with tile.TileContext(nc) as tc, ExitStack() as ctx:
        consts = ctx.enter_context(tc.tile_pool(name="consts", bufs=1))
        wt_pool = ctx.enter_context(tc.tile_pool(name="wt", bufs=1))
        wraw_pool = ctx.enter_context(tc.tile_pool(name="wraw", bufs=2))
        xin_pool = ctx.enter_context(tc.tile_pool(name="xin", bufs=4))
        xt_pool = ctx.enter_context(tc.tile_pool(name="xt", bufs=3))
        qk_pool = ctx.enter_context(tc.tile_pool(name="qk", bufs=2))
        vh_pool = ctx.enter_context(tc.tile_pool(name="vh", bufs=2))
        st_pool = ctx.enter_context(tc.tile_pool(name="st", bufs=2))
        p_pool = ctx.enter_context(tc.tile_pool(name="pp", bufs=16))
        dn_pool = ctx.enter_context(tc.tile_pool(name="dn", bufs=2))
        o_pool = ctx.enter_context(tc.tile_pool(name="oo", bufs=2))
        ps_mm = ctx.enter_context(tc.tile_pool(name="psmm", bufs=2, space="PSUM"))
        ps_s = ctx.enter_context(tc.tile_pool(name="pss", bufs=2, space="PSUM"))
        ps_o = ctx.enter_context(tc.tile_pool(name="pso", bufs=2, space="PSUM"))

        ident = consts.tile([P, P], FP16)
        nc.sync.dma_start(ident[:], id16_d)
        ident32 = consts.tile([P, P], FP32)
        nc.sync.dma_start(ident32[:], id32_d)
        sel = consts.tile([P, H * D], FP16)
        nc.sync.dma_start(sel[:], sel_d)
        bo_t = consts.tile([P, E], FP32)
        nc.sync.dma_start(bo_t[:], bo_d)

        early_units = []
        wts = {}

        def emit_weights():
            for wname, w_d in [("q", wq_d), ("k", wk_d), ("v", wv_d), ("o", wo_d)]:
                w_raw = wraw_pool.tile([P, EPO, E], FP32, tag="wraw",
                                       name=f"wraw_{wname}")
                nc.sync.dma_start(w_raw[:], w_d.rearrange("(fo fi) e -> fi fo e", fi=P))
                wt = wt_pool.tile([P, EPO, E], FP16, tag=f"wt_{wname}",
                                  name=f"wt_{wname}")
                for epo in range(EPO):
                    ps = ps_mm.tile([P, E], FP32, tag="mm", name=f"wps_{wname}_{epo}")
                    for fpo in range(EPO):
                        nc.tensor.transpose(
                            ps[:, fpo * P:(fpo + 1) * P],
                            w_raw[:, fpo, epo * P:(epo + 1) * P],
                            ident32[:],
                        )
                    if wname == "q":
                        nc.vector.tensor_scalar_mul(wt[:, epo, :], ps[:], 1.0 / np.sqrt(D))
                    else:
                        nc.vector.tensor_copy(wt[:, epo, :], ps[:])
                wts[wname] = wt

        xts = [dict() for _ in range(NB)]
        qkts = [dict() for _ in range(NB)]
        vhs = [None] * NB
        stages = [None] * NB
        denoms = [None] * NB
        recips = [None] * NB

        def ensure_xt(b, tname):
            if tname not in xts[b]:
                xts[b][tname] = xt_pool.tile([P, EPO, L], FP16, tag="xt", name=f"xt_{b}_{tname}")
            return xts[b][tname]

        alt_state = [0]

        def pick_act(on_act):
            if on_act == "alt":
                alt_state[0] ^= 1
                return bool(alt_state[0])
            return on_act

        def emit_xtile(b, tname, tt, on_act):
            on_act = pick_act(on_act)
            xt = ensure_xt(b, tname)
            xin = xin_pool.tile([P, E], FP32, tag="xin")
            nc.sync.dma_start(xin[:], x_d[tname][b, tt * P:(tt + 1) * P, :])
            xin_b = xin_pool.tile([P, E], FP16, tag="xinb")
            nc.vector.tensor_copy(xin_b[:], xin[:])
            ps = ps_mm.tile([P, E], FP16, tag="mm")
            for epo in range(EPO):
                nc.tensor.transpose(
                    ps[:, epo * P:(epo + 1) * P],
                    xin_b[:, epo * P:(epo + 1) * P],
                    ident[:],
                )
            dst = xt[:, :, tt * P:(tt + 1) * P]
            src = ps[:].rearrange("p (epo t) -> p epo t", epo=EPO)
            if on_act:
                nc.scalar.copy(dst, src)
            else:
                nc.vector.tensor_copy(dst, src)

        def emit_qk_proj(b, tname, fpo, tch, on_act):
            wt = wts[tname]
            xt = xts[b][tname]
            if tname not in qkts[b]:
                qkts[b][tname] = qk_pool.tile([P, EPO, L], FP16, tag=f"ht_{tname}", name=f"ht_{b}_{tname}")
            ht = qkts[b][tname]
            ps = ps_mm.tile([P, E], FP32, tag="mm")
            for epo in range(EPO):
                nc.tensor.matmul(
                    ps[:],
                    wt[:, epo, fpo * P:(fpo + 1) * P],
                    xt[:, epo, tch * E:(tch + 1) * E],
                    start=(epo == 0),
                    stop=(epo == EPO - 1),
                )
            dst = ht[:, fpo, tch * E:(tch + 1) * E]
            if on_act:
                nc.scalar.copy(dst, ps[:])
            else:
                nc.vector.tensor_copy(dst, ps[:])

        def emit_vh_init(b):
            vh = vh_pool.tile([P, TT, H, D + 1], FP16, tag="vh")
            vhs[b] = vh
            nc.vector.memset(vh[:, :, :, D:D + 1], 1.0)

        def emit_vh(b, tt, on_act):
            vh = vhs[b]
            wt = wts["v"]
            xt = xts[b]["v"]
            ps = ps_mm.tile([P, E], FP32, tag="mm")
            for epo in range(EPO):
                nc.tensor.matmul(
                    ps[:],
                    xt[:, epo, tt * P:(tt + 1) * P],
                    wt[:, epo, :],
                    start=(epo == 0),
                    stop=(epo == EPO - 1),
                )
            dst = vh[:, tt, :, 0:D]
            src = ps[:].rearrange("p (h d) -> p h d", h=H)
            if on_act:
                nc.scalar.copy(dst, src)
            else:
                nc.vector.tensor_copy(dst, src)

        queue = []
        done_labels = set()

        def _run(entry):
            label, fn = entry
            fn()
            if label is not None:
                done_labels.add(label)

        def pump(n):
            for _ in range(min(n, len(queue))):
                _run(queue.pop(0))

        def drain_until(label):
            while label not in done_labels and queue:
                _run(queue.pop(0))

        def emit_s_exp_pair(b, hpo, lt):
            qht, kht = qkts[b]["q"], qkts[b]["k"]
            pss = [ps_s.tile([P, L], FP32, tag="s", name=f"pss_{b}_{hpo}_{lt}_{i}") for i in range(2)]
            for ch in range(L // E):
                for hh in range(2):
                    hoff = D * hh
                    nc.tensor.matmul(
                        pss[hh][:, ch * E:(ch + 1) * E],
                        kht[hoff:hoff + D, hpo, lt * P:(lt + 1) * P],
                        qht[hoff:hoff + D, hpo, ch * E:(ch + 1) * E],
                        start=True,
                        stop=True,
                    )
            pts = []
            for hh in range(2):
                pt = p_pool.tile([P, L], FP16, tag="p")
                nc.scalar.activation(pt[:], pss[hh][:], AF.Exp)
                pts.append(pt)
            return pts

        def emit_av(b, h, pts_lt, interleave=()):
            vh = vhs[b]
            stage = stages[b]
            denom = denoms[b]
            hpo, hoff = h // 2, D * (h % 2)
            inter = list(interleave)
            for ch in range(L // E):
                pso = ps_o.tile([D + 1, E], FP32, tag="o")
                for lt in range(TT):
                    nc.tensor.matmul(
                        pso[:],
                        vh[:, lt, h, :],
                        pts_lt[lt][:, ch * E:(ch + 1) * E],
                        start=(lt == 0),
                        stop=(lt == TT - 1),
                    )
                nc.vector.tensor_copy(
                    stage[hoff:hoff + D, hpo, ch * E:(ch + 1) * E], pso[0:D, :]
                )
                nc.vector.tensor_copy(
                    denom[32 * (h % 4):32 * (h % 4) + 1, h // 4, ch * E:(ch + 1) * E],
                    pso[D:D + 1, :],
                )
                if inter:
                    inter.pop(0)()

        def emit_recip(b, half):
            denom = denoms[b]
            if recips[b] is None:
                recips[b] = (dn_pool.tile([P, 2, L], FP32, tag="dnr32", bufs=1,
                                          name=f"r32_{b}"),
                             dn_pool.tile([P, 2, L], FP16, tag="dnr", bufs=2,
                                          name=f"recip_{b}"))
            r32, recip = recips[b]
            nc.vector.reciprocal_approx_fast(r32[:, half, :], denom[:, half, :])
            nc.vector.tensor_copy(recip[:, half, :], r32[:, half, :])

        def emit_norm_head(b, h):
            stage = stages[b]
            recip = recips[b][1]
            hpo, hoff = h // 2, D * (h % 2)
            for ch in range(L // E):
                psb = ps_o.tile([D, E], FP32, tag="o")
                nc.tensor.matmul(
                    psb[:],
                    sel[:, h * D:(h + 1) * D],
                    recip[:, h // 4, ch * E:(ch + 1) * E],
                    start=True,
                    stop=True,
                )
                nc.vector.tensor_tensor(
                    stage[hoff:hoff + D, hpo, ch * E:(ch + 1) * E],
                    psb[:],
                    stage[hoff:hoff + D, hpo, ch * E:(ch + 1) * E],
                    ALU.mult,
                )

        def emit_outproj(b, tt):
            stage = stages[b]
            wt = wts["o"]
            ps = ps_mm.tile([P, E], FP32, tag="mm")
            for epo in range(EPO):
                nc.tensor.matmul(
                    ps[:],
                    stage[:, epo, tt * P:(tt + 1) * P],
                    wt[:, epo, :],
                    start=(epo == 0),
                    stop=(epo == EPO - 1),
                )
            ot = o_pool.tile([P, E], FP32, tag="ot")
            nc.vector.tensor_tensor(ot[:], ps[:], bo_t[:], ALU.add)
            nc.gpsimd.dma_start(out_d[b, tt * P:(tt + 1) * P, :], ot[:])

        def prep_phase1_units(b, on_act):
            u = []
            for tname in ("k", "q"):
                for tt in range(TT):
                    u.append((None, lambda b=b, t=tname, tt=tt: emit_xtile(b, t, tt, on_act)))
            for i, (tname, tch) in enumerate(
                    [(t, c) for t in ("k", "q") for c in range(L // E)]):
                lbl = ("proj", b, 0) if i == 3 else None
                u.append((lbl, lambda b=b, t=tname, tch=tch: emit_qk_proj(b, t, 0, tch, on_act)))
            return u

        def prep_phase2_units(b, on_act):
            u = [(None, lambda b=b: emit_vh_init(b))]
            for tt in range(TT):
                u.append((None, lambda b=b, tt=tt: emit_xtile(b, "v", tt, on_act)))
            for tt in range(TT):
                lbl = ("vh", b) if tt == TT - 1 else None
                u.append((lbl, lambda b=b, tt=tt: emit_vh(b, tt, on_act)))
            return u

        def proj_units(b, fpo, on_act):
            u = []
            for i, (tname, tch) in enumerate(
                    [(t, c) for t in ("k", "q") for c in range(L // E)]):
                lbl = ("proj", b, fpo) if i == 3 else None
                u.append((lbl, lambda b=b, t=tname, f=fpo, tch=tch: emit_qk_proj(b, t, f, tch, on_act)))
            return u

        def attn_pair(b, hpo, tail_units):
            if stages[b] is None:
                stages[b] = st_pool.tile([P, EPO, L], FP16, tag="st", name=f"stage_{b}")
                denoms[b] = dn_pool.tile([P, 2, L], FP32, tag="dn", name=f"denom_{b}", bufs=2)
                nc.vector.memset(denoms[b][:], 1.0)
            drain_until(("proj", b, hpo))
            pts_pair = [[], []]
            for lt in range(TT):
                pts = emit_s_exp_pair(b, hpo, lt)
                pts_pair[0].append(pts[0])
                pts_pair[1].append(pts[1])
                if queue:
                    pump(4)
                elif tail_units:
                    for _ in range(2):
                        if tail_units:
                            tail_units.pop(0)()
            drain_until(("vh", b))
            for hh in range(2):
                h = 2 * hpo + hh
                inter = []
                for _ in range(2):
                    if queue:
                        inter.append(lambda e=queue.pop(0): _run(e))
                    elif tail_units:
                        inter.append(tail_units.pop(0))
                emit_av(b, h, pts_pair[hh], interleave=inter)

        emit_weights()
        for u in prep_phase1_units(0, on_act="alt"):
            _run(u)
        queue.extend(prep_phase2_units(0, on_act=True))
        for fpo in range(1, EPO):
            queue.extend(proj_units(0, fpo, on_act=True))
        queue.extend(prep_phase1_units(1, on_act=False))
        queue.extend(prep_phase2_units(1, on_act=False))
        for fpo in range(1, EPO):
            queue.extend(proj_units(1, fpo, on_act=False))

        tails = []
        for b in range(NB):
            for hpo in range(H // 2):
                attn_pair(b, hpo, tails)
                if hpo == 1:
                    tails += [lambda b=b: emit_recip(b, 0)]
                    tails += [lambda b=b, h=h: emit_norm_head(b, h)
                              for h in range(4)]
                elif hpo == H // 2 - 1:
                    tails += [lambda b=b: emit_recip(b, 1)]
                    tails += [lambda b=b, h=h: emit_norm_head(b, h)
                              for h in range(4, H)]
                    tails += [lambda b=b, tt=tt: emit_outproj(b, tt)
                              for tt in range(TT)]
        for u in tails:
            u()

    nc.compile()
    return nc


_COMPILED = None


def _get_compiled():
    global _COMPILED
    if _COMPILED is None:
        _COMPILED = build()
    return _COMPILED


def kernel(q, k, v, Wq, Wk, Wv, Wo, bo):
    import numpy as _np

    q = _np.ascontiguousarray(_np.asarray(q, dtype=_np.float32))
    k = _np.ascontiguousarray(_np.asarray(k, dtype=_np.float32))
    v = _np.ascontiguousarray(_np.asarray(v, dtype=_np.float32))
    Wq = _np.ascontiguousarray(_np.asarray(Wq, dtype=_np.float32))
    Wk = _np.ascontiguousarray(_np.asarray(Wk, dtype=_np.float32))
    Wv = _np.ascontiguousarray(_np.asarray(Wv, dtype=_np.float32))
    Wo = _np.ascontiguousarray(_np.asarray(Wo, dtype=_np.float32))
    bo = _np.asarray(bo, dtype=_np.float32)

    nc = _get_compiled()
    ident16, ident32, sel2 = host_constants()
    bo_bcast = _np.ascontiguousarray(_np.broadcast_to(bo, (P, E)))
    n_cores = 8
    in_maps = []
    for c in range(n_cores):
        in_maps.append({
            "q": _np.ascontiguousarray(q[c * NB:(c + 1) * NB]),
            "k": _np.ascontiguousarray(k[c * NB:(c + 1) * NB]),
            "v": _np.ascontiguousarray(v[c * NB:(c + 1) * NB]),
            "Wq": Wq, "Wk": Wk, "Wv": Wv, "Wo": Wo,
            "bo_bcast": bo_bcast, "ident16": ident16, "ident32": ident32,
            "sel2": sel2,
        })

    from concourse.bass_utils import run_bass_kernel_spmd
    res = run_bass_kernel_spmd(nc, in_maps, core_ids=list(range(n_cores)))
    out = _np.concatenate([res.results[c]["out"] for c in range(n_cores)], axis=0)
    return out.astype(_np.float32)


# revision 14
# speedup vs baseline: 1.0188x; 1.0188x over previous
import numpy as np
import concourse.bass as bass
import concourse.mybir as mybir
import concourse.tile as tile
from contextlib import ExitStack
from concourse import bacc

P = 128
L = 1024
E = 512
H = 8
D = 64
NB = 2
TT = L // P
EPO = E // P
FP32 = mybir.dt.float32
FP16 = mybir.dt.float16
AF = mybir.ActivationFunctionType
ALU = mybir.AluOpType


def host_constants():
    ident16 = np.eye(P, dtype=np.float16)
    ident32 = np.eye(P, dtype=np.float32)
    sel2 = np.zeros((P, H * D), np.float16)
    for h in range(H):
        sel2[32 * (h % 4), h * D:(h + 1) * D] = 1.0
    return ident16, ident32, sel2


def build(debug=False):
    nc = bacc.Bacc("TRN2", target_bir_lowering=False, debug=debug)
    q_d = nc.dram_tensor("q", [NB, L, E], FP32, kind="ExternalInput").ap()
    k_d = nc.dram_tensor("k", [NB, L, E], FP32, kind="ExternalInput").ap()
    v_d = nc.dram_tensor("v", [NB, L, E], FP32, kind="ExternalInput").ap()
    wq_d = nc.dram_tensor("Wq", [E, E], FP32, kind="ExternalInput").ap()
    wk_d = nc.dram_tensor("Wk", [E, E], FP32, kind="ExternalInput").ap()
    wv_d = nc.dram_tensor("Wv", [E, E], FP32, kind="ExternalInput").ap()
    wo_d = nc.dram_tensor("Wo", [E, E], FP32, kind="ExternalInput").ap()
    bo_d = nc.dram_tensor("bo_bcast", [P, E], FP32, kind="ExternalInput").ap()
    id16_d = nc.dram_tensor("ident16", [P, P], FP16, kind="ExternalInput").ap()
    id32_d = nc.dram_tensor("ident32", [P, P], FP32, kind="ExternalInput").ap()
    sel_d = nc.dram_tensor("sel2", [P, H * D], FP16, kind="ExternalInput").ap()
    out_d = nc.dram_tensor("out", [NB, L, E], FP32, kind="ExternalOutput").ap()
    x_d = {"q": q_d, "k": k_d, "v": v_d}

    with tile.TileContext(nc) as tc, ExitStack() as ctx:
        consts = ctx.enter_context(tc.tile_pool(name="consts", bufs=1))
        wt_pool = ctx.enter_context(tc.tile_pool(name="wt", bufs=1))
        xin_pool = ctx.enter_context(tc.tile_pool(name="xin", bufs=4))
        xt_pool = ctx.enter_context(tc.tile_pool(name="xt", bufs=3))
        qk_pool = ctx.enter_context(tc.tile_pool(name="qk", bufs=2))
        vh_pool = ctx.enter_context(tc.tile_pool(name="vh", bufs=2))
        st_pool = ctx.enter_context(tc.tile_pool(name="st", bufs=2))
        p_pool = ctx.enter_context(tc.tile_pool(name="pp", bufs=24))
        dn_pool = ctx.enter_context(tc.tile_pool(name="dn", bufs=2))
        o_pool = ctx.enter_context(tc.tile_pool(name="oo", bufs=2))
        ps_mm = ctx.enter_context(tc.tile_pool(name="psmm", bufs=2, space="PSUM"))
        ps_s = ctx.enter_context(tc.tile_pool(name="pss", bufs=2, space="PSUM"))
        ps_o = ctx.enter_context(tc.tile_pool(name="pso", bufs=2, space="PSUM"))

        ident = consts.tile([P, P], FP16)
        nc.sync.dma_start(ident[:], id16_d)
        ident32 = consts.tile([P, P], FP32)
        nc.sync.dma_start(ident32[:], id32_d)
        sel = consts.tile([P, H * D], FP16)
        nc.sync.dma_start(sel[:], sel_d)
        bo_t = consts.tile([P, E], FP32)
        nc.sync.dma_start(bo_t[:], bo_d)

        early_units = []
        wts = {}

        def emit_weights():
            for wname, w_d in [("q", wq_d), ("k", wk_d), ("v", wv_d), ("o", wo_d)]:
                w_raw = xt_pool.tile([P, EPO, E], FP32, tag="xt",
                                     name=f"wraw_{wname}")
                nc.sync.dma_start(w_raw[:], w_d.rearrange("(fo fi) e -> fi fo e", fi=P))
                wt = wt_pool.tile([P, EPO, E], FP16, tag=f"wt_{wname}",
                                  name=f"wt_{wname}")
                for epo in range(EPO):
                    ps = ps_mm.tile([P, E], FP32, tag="mm", name=f"wps_{wname}_{epo}")
                    for fpo in range(EPO):
                        nc.tensor.transpose(
                            ps[:, fpo * P:(fpo + 1) * P],
                            w_raw[:, fpo, epo * P:(epo + 1) * P],
                            ident32[:],
                        )
                    if wname == "q":
                        nc.vector.tensor_scalar_mul(wt[:, epo, :], ps[:], 1.0 / np.sqrt(D))
                    else:
                        nc.vector.tensor_copy(wt[:, epo, :], ps[:])
                wts[wname] = wt

        xts = [dict() for _ in range(NB)]
        qkts = [dict() for _ in range(NB)]
        vhs = [None] * NB
        stages = [None] * NB
        denoms = [None] * NB
        recips = [None] * NB

        def ensure_xt(b, tname):
            if tname not in xts[b]:
                xts[b][tname] = xt_pool.tile([P, EPO, L], FP16, tag="xt", name=f"xt_{b}_{tname}")
            return xts[b][tname]

        alt_state = [0]

        def pick_act(on_act):
            if on_act == "alt":
                alt_state[0] ^= 1
                return bool(alt_state[0])
            return on_act

        def emit_xtile(b, tname, tt, on_act):
            on_act = pick_act(on_act)
            xt = ensure_xt(b, tname)
            xin = xin_pool.tile([P, E], FP32, tag="xin")
            nc.sync.dma_start(xin[:], x_d[tname][b, tt * P:(tt + 1) * P, :])
            xin_b = xin_pool.tile([P, E], FP16, tag="xinb")
            nc.vector.tensor_copy(xin_b[:], xin[:])
            ps = ps_mm.tile([P, E], FP16, tag="mm")
            for epo in range(EPO):
                nc.tensor.transpose(
                    ps[:, epo * P:(epo + 1) * P],
                    xin_b[:, epo * P:(epo + 1) * P],
                    ident[:],
                )
            dst = xt[:, :, tt * P:(tt + 1) * P]
            src = ps[:].rearrange("p (epo t) -> p epo t", epo=EPO)
            if on_act:
                nc.scalar.copy(dst, src)
            else:
                nc.vector.tensor_copy(dst, src)

        def emit_qk_proj(b, tname, fpo, tch, on_act):
            on_act = pick_act(on_act)
            wt = wts[tname]
            xt = xts[b][tname]
            if tname not in qkts[b]:
                qkts[b][tname] = qk_pool.tile([P, EPO, L], FP16, tag=f"ht_{tname}", name=f"ht_{b}_{tname}")
            ht = qkts[b][tname]
            ps = ps_mm.tile([P, E], FP32, tag="mm")
            for epo in range(EPO):
                nc.tensor.matmul(
                    ps[:],
                    wt[:, epo, fpo * P:(fpo + 1) * P],
                    xt[:, epo, tch * E:(tch + 1) * E],
                    start=(epo == 0),
                    stop=(epo == EPO - 1),
                )
            dst = ht[:, fpo, tch * E:(tch + 1) * E]
            if on_act:
                nc.scalar.copy(dst, ps[:])
            else:
                nc.vector.tensor_copy(dst, ps[:])

        def emit_vh_init(b):
            vh = vh_pool.tile([P, TT, H, D + 1], FP16, tag="vh")
            vhs[b] = vh
            nc.vector.memset(vh[:, :, :, D:D + 1], 1.0)

        def emit_vh(b, tt, on_act):
            vh = vhs[b]
            wt = wts["v"]
            xt = xts[b]["v"]
            ps = ps_mm.tile([P, E], FP32, tag="mm")
            for epo in range(EPO):
                nc.tensor.matmul(
                    ps[:],
                    xt[:, epo, tt * P:(tt + 1) * P],
                    wt[:, epo, :],
                    start=(epo == 0),
                    stop=(epo == EPO - 1),
                )
            dst = vh[:, tt, :, 0:D]
            src = ps[:].rearrange("p (h d) -> p h d", h=H)
            if on_act:
                nc.scalar.copy(dst, src)
            else:
                nc.vector.tensor_copy(dst, src)

        queue = []
        done_labels = set()

        def _run(entry):
            label, fn = entry
            fn()
            if label is not None:
                done_labels.add(label)

        def pump(n):
            for _ in range(min(n, len(queue))):
                _run(queue.pop(0))

        def drain_until(label):
            while label not in done_labels and queue:
                _run(queue.pop(0))

        def emit_s_exp_pair(b, hpo, lt):
            qht, kht = qkts[b]["q"], qkts[b]["k"]
            pss = [ps_s.tile([P, L], FP32, tag="s", name=f"pss_{b}_{hpo}_{lt}_{i}") for i in range(2)]
            for ch in range(L // E):
                for hh in range(2):
                    hoff = D * hh
                    nc.tensor.matmul(
                        pss[hh][:, ch * E:(ch + 1) * E],
                        kht[hoff:hoff + D, hpo, lt * P:(lt + 1) * P],
                        qht[hoff:hoff + D, hpo, ch * E:(ch + 1) * E],
                        start=True,
                        stop=True,
                    )
            pts = []
            for hh in range(2):
                pt = p_pool.tile([P, L], FP16, tag="p")
                nc.scalar.activation(pt[:], pss[hh][:], AF.Exp)
                pts.append(pt)
            return pts

        def emit_av(b, h, pts_lt, interleave=()):
            vh = vhs[b]
            stage = stages[b]
            denom = denoms[b]
            hpo, hoff = h // 2, D * (h % 2)
            inter = list(interleave)
            for ch in range(L // E):
                pso = ps_o.tile([D + 1, E], FP32, tag="o")
                for lt in range(TT):
                    nc.tensor.matmul(
                        pso[:],
                        vh[:, lt, h, :],
                        pts_lt[lt][:, ch * E:(ch + 1) * E],
                        start=(lt == 0),
                        stop=(lt == TT - 1),
                    )
                nc.vector.tensor_copy(
                    stage[hoff:hoff + D, hpo, ch * E:(ch + 1) * E], pso[0:D, :]
                )
                nc.vector.tensor_copy(
                    denom[32 * (h % 4):32 * (h % 4) + 1, h // 4, ch * E:(ch + 1) * E],
                    pso[D:D + 1, :],
                )
                if inter:
                    inter.pop(0)()

        def emit_recip(b, half):
            denom = denoms[b]
            if recips[b] is None:
                recips[b] = dn_pool.tile([P, 2, L], FP16, tag="dnr", bufs=2,
                                         name=f"recip_{b}")
            recip = recips[b]
            r32 = dn_pool.tile([P, L], FP32, tag="dnr32", bufs=1,
                               name=f"r32_{b}_{half}")
            nc.vector.reciprocal_approx_fast(r32[:], denom[:, half, :])
            nc.vector.tensor_copy(recip[:, half, :], r32[:])

        def emit_norm_head(b, h):
            stage = stages[b]
            recip = recips[b]
            hpo, hoff = h // 2, D * (h % 2)
            for ch in range(L // E):
                psb = ps_o.tile([D, E], FP32, tag="o")
                nc.tensor.matmul(
                    psb[:],
                    sel[:, h * D:(h + 1) * D],
                    recip[:, h // 4, ch * E:(ch + 1) * E],
                    start=True,
                    stop=True,
                )
                nc.vector.tensor_tensor(
                    stage[hoff:hoff + D, hpo, ch * E:(ch + 1) * E],
                    psb[:],
                    stage[hoff:hoff + D, hpo, ch * E:(ch + 1) * E],
                    ALU.mult,
                )

        def emit_outproj(b, tt):
            stage = stages[b]
            wt = wts["o"]
            ps = ps_mm.tile([P, E], FP32, tag="mm")
            for epo in range(EPO):
                nc.tensor.matmul(
                    ps[:],
                    stage[:, epo, tt * P:(tt + 1) * P],
                    wt[:, epo, :],
                    start=(epo == 0),
                    stop=(epo == EPO - 1),
                )
            ot = o_pool.tile([P, E], FP32, tag="ot")
            nc.vector.tensor_tensor(ot[:], ps[:], bo_t[:], ALU.add)
            nc.gpsimd.dma_start(out_d[b, tt * P:(tt + 1) * P, :], ot[:])

        def prep_phase1_units(b, on_act):
            u = []
            for tname in ("k", "q"):
                for tt in range(TT):
                    u.append((None, lambda b=b, t=tname, tt=tt: emit_xtile(b, t, tt, on_act)))
            for i, (tname, tch) in enumerate(
                    [(t, c) for t in ("k", "q") for c in range(L // E)]):
                lbl = ("proj", b, 0) if i == 3 else None
                u.append((lbl, lambda b=b, t=tname, tch=tch: emit_qk_proj(b, t, 0, tch, on_act)))
            return u

        def prep_phase2_units(b, on_act):
            u = [(None, lambda b=b: emit_vh_init(b))]
            for tt in range(TT):
                u.append((None, lambda b=b, tt=tt: emit_xtile(b, "v", tt, on_act)))
            for tt in range(TT):
                lbl = ("vh", b) if tt == TT - 1 else None
                u.append((lbl, lambda b=b, tt=tt: emit_vh(b, tt, on_act)))
            return u

        def proj_units(b, fpo, on_act):
            u = []
            for i, (tname, tch) in enumerate(
                    [(t, c) for t in ("k", "q") for c in range(L // E)]):
                lbl = ("proj", b, fpo) if i == 3 else None
                u.append((lbl, lambda b=b, t=tname, f=fpo, tch=tch: emit_qk_proj(b, t, f, tch, on_act)))
            return u

        def attn_pair(b, hpo, tail_units):
            if stages[b] is None:
                stages[b] = st_pool.tile([P, EPO, L], FP16, tag="st", name=f"stage_{b}")
                denoms[b] = dn_pool.tile([P, 2, L], FP32, tag="dn", name=f"denom_{b}", bufs=2)
                nc.vector.memset(denoms[b][:], 1.0)
            drain_until(("proj", b, hpo))
            pts_pair = [[], []]
            for lt in range(TT):
                pts = emit_s_exp_pair(b, hpo, lt)
                pts_pair[0].append(pts[0])
                pts_pair[1].append(pts[1])
                if queue:
                    pump(4)
                elif tail_units:
                    for _ in range(2):
                        if tail_units:
                            tail_units.pop(0)()
            drain_until(("vh", b))
            for hh in range(2):
                h = 2 * hpo + hh
                inter = []
                for _ in range(2):
                    if queue:
                        inter.append(lambda e=queue.pop(0): _run(e))
                    elif tail_units:
                        inter.append(tail_units.pop(0))
                emit_av(b, h, pts_pair[hh], interleave=inter)

        emit_weights()
        for u in prep_phase1_units(0, on_act="alt"):
            _run(u)
        for fpo in range(1, EPO):
            for u in proj_units(0, fpo, on_act="alt"):
                _run(u)
        queue.extend(prep_phase2_units(0, on_act=True))
        queue.extend(prep_phase1_units(1, on_act=False))
        queue.extend(prep_phase2_units(1, on_act=False))
        for fpo in range(1, EPO):
            queue.extend(proj_units(1, fpo, on_act=False))

        tails = []
        for b in range(NB):
            for hpo in range(H // 2):
                attn_pair(b, hpo, tails)
                if hpo == 1:
                    tails += [lambda b=b: emit_recip(b, 0)]
                    tails += [lambda b=b, h=h: emit_norm_head(b, h)
                              for h in range(4)]
                elif hpo == H // 2 - 1:
                    tails += [lambda b=b: emit_recip(b, 1)]
                    tails += [lambda b=b, h=h: emit_norm_head(b, h)
                              for h in range(4, H)]
                    tails += [lambda b=b, tt=tt: emit_outproj(b, tt)
                              for tt in range(TT)]
        for u in tails:
            u()

    nc.compile()
    return nc


_COMPILED = None


def _get_compiled():
    global _COMPILED
    if _COMPILED is None:
        _COMPILED = build()
    return _COMPILED


def kernel(q, k, v, Wq, Wk, Wv, Wo, bo):
    import numpy as _np

    q = _np.ascontiguousarray(_np.asarray(q, dtype=_np.float32))
    k = _np.ascontiguousarray(_np.asarray(k, dtype=_np.float32))
    v = _np.ascontiguousarray(_np.asarray(v, dtype=_np.float32))
    Wq = _np.ascontiguousarray(_np.asarray(Wq, dtype=_np.float32))
    Wk = _np.ascontiguousarray(_np.asarray(Wk, dtype=_np.float32))
    Wv = _np.ascontiguousarray(_np.asarray(Wv, dtype=_np.float32))
    Wo = _np.ascontiguousarray(_np.asarray(Wo, dtype=_np.float32))
    bo = _np.asarray(bo, dtype=_np.float32)

    nc = _get_compiled()
    ident16, ident32, sel2 = host_constants()
    bo_bcast = _np.ascontiguousarray(_np.broadcast_to(bo, (P, E)))
    n_cores = 8
    in_maps = []
    for c in range(n_cores):
        in_maps.append({
            "q": _np.ascontiguousarray(q[c * NB:(c + 1) * NB]),
            "k": _np.ascontiguousarray(k[c * NB:(c + 1) * NB]),
            "v": _np.ascontiguousarray(v[c * NB:(c + 1) * NB]),
            "Wq": Wq, "Wk": Wk, "Wv": Wv, "Wo": Wo,
            "bo_bcast": bo_bcast, "ident16": ident16, "ident32": ident32,
            "sel2": sel2,
        })

    from concourse.bass_utils import run_bass_kernel_spmd
    res = run_bass_kernel_spmd(nc, in_maps, core_ids=list(range(n_cores)))
    out = _np.concatenate([res.results[c]["out"] for c in range(n_cores)], axis=0)
    return out.astype(_np.float32)
